# revision 57
# baseline (speedup 1.0000x reference)
"""Causal self-attention (RoPE) Trainium2 Bass kernel, SPMD over 8 NeuronCores.

Sharding: core i -> batch b = i // 4, head group hg = i % 4 (4 heads each).
Per core: QKV projections (f32r matmuls), RoPE via rotated-weight projections,
scores computed transposed [k, q] (softmax denominator via ones-column in V),
causal block skipping, out-proj partial. Host sums the 4 partials per batch.
"""
import numpy as np
import ml_dtypes

import concourse.mybir as mybir
import concourse.tile as tile
from concourse import bacc
from concourse.bass_utils import run_bass_kernel_spmd

B, S, D = 2, 2048, 1024
H, HD = 16, 64
NCORES = 8
GROUPS = NCORES // B          # 4 tensor-parallel cores per batch
HLOC = H // GROUPS            # 4 heads per core
FLOC = HLOC * HD              # 256 local features
P = 128
SC = 512                      # s-chunk (q-chunk) width
NCH = S // SC                 # 4 chunks
KBLK = S // P                 # 16 key blocks
DKB = D // P                  # 8 contraction blocks for projections
NEG = -200.0                  # clamped mask value; exp(-200+s) == 0 in fp32

F32 = mybir.dt.float32
F32R = mybir.dt.float32r
BF16 = mybir.dt.bfloat16
AF = mybir.ActivationFunctionType

# stage dtypes (bf16 streams 2 cols/cycle on PE + fast weight load; f32r is
# ~11-bit-mantissa fp32 at 1 col/cycle)
PROJ_BF16 = True    # xT/wqk/wro/wv/wo/aT (projection + out-proj matmuls)
ATT_BF16 = True     # qT/kT/v_aug/pT/ident/mtri/mask (attention matmuls)
GP_BCAST = True     # softmax-recip broadcast on GpSimd (else PE mm + ACT copy)
GP_ROPE_ADD = False # RoPE t1+t2 add on DVE (gpsimd would force library reloads)
ROPE_PE = True      # rotate_half via one PE permutation matmul instead of a
                    # second full rotated-weight projection (saves 7 MMs/m-tile)

_CACHE: dict = {}
_LAST_RESULTS = None
_LAST_IN_MAPS = None


def _build(causal: bool, has_mask: bool, has_bias: bool, repeat: int = 1):
    nc = bacc.Bacc("TRN2", target_bir_lowering=False, debug=False,
                   num_devices=NCORES)
    dp = nc.declare_dram_parameter
    pdt = BF16 if PROJ_BF16 else F32R
    adt = BF16 if ATT_BF16 else F32R
    io = {
        # x^T permuted [p, kb, s] so each chunk loads with ONE dma
        "xT":   dp("xT",   [P, DKB, S],   pdt, isOutput=False),
        "wqk":  dp("wqk",  [D, 2 * FLOC], pdt, isOutput=False),
        "wv":   dp("wv",   [D, FLOC],     pdt, isOutput=False),
        "wo":   dp("wo",   [FLOC, D],     pdt, isOutput=False),
        "cos2": dp("cos2", [P, S],        F32,  isOutput=False),
        "sin2": dp("sin2", [P, S],        F32,  isOutput=False),
        # y stored [c, p, st*D+d] (token s = c*SC + st*P + p), bf16 partials
        # summed on host in f32
        "y":    dp("y",    [NCH, P, (SC // P) * D], BF16, isOutput=True),
    }
    if ROPE_PE:
        io["rotM"] = dp("rotM", [P, P], adt, isOutput=False)
    else:
        io["wro"] = dp("wro", [D, 2 * FLOC], pdt, isOutput=False)
    if causal:
        io["tri01"] = dp("tri01", [P, P], adt, isOutput=False)
    elif has_mask:
        io["ident"] = dp("ident", [P, P], adt, isOutput=False)
        io["maskT"] = dp("maskT", [S, S], adt, isOutput=False)
    if has_bias:
        io["bqk"] = dp("bqk", [1, 2 * FLOC], F32R, isOutput=False)
        io["bro"] = dp("bro", [1, 2 * FLOC], F32R, isOutput=False)
        io["bv"] = dp("bv", [1, FLOC], F32R, isOutput=False)

    with tile.TileContext(nc) as tc, \
         nc.allow_low_precision(reason="float32r rounding for PE operands"):
        _emit(nc, tc, io, causal, has_mask, has_bias, repeat)
    nc.finalize()
    return nc


def _emit(nc, tc, io, causal, has_mask, has_bias, repeat=1):
    pdt = BF16 if PROJ_BF16 else F32R
    adt = BF16 if ATT_BF16 else F32R
    from contextlib import ExitStack
    ctx = ExitStack()
    with ctx:
        wpool = ctx.enter_context(tc.tile_pool(name="weights", bufs=1))
        xpool = ctx.enter_context(tc.tile_pool(name="xt", bufs=2))
        qkpool = ctx.enter_context(tc.tile_pool(name="qk", bufs=1))
        vpool = ctx.enter_context(tc.tile_pool(name="v", bufs=1))
        apool = ctx.enter_context(tc.tile_pool(name="aT", bufs=1))
        tmppool = ctx.enter_context(tc.tile_pool(name="tmp", bufs=2))
        ptpool = ctx.enter_context(tc.tile_pool(name="pT", bufs=3))
        smpool = ctx.enter_context(tc.tile_pool(name="small", bufs=2))
        ypool = ctx.enter_context(tc.tile_pool(name="y", bufs=2))
        mmps = ctx.enter_context(tc.tile_pool(name="mmps", bufs=2, space="PSUM"))
        scps = ctx.enter_context(tc.tile_pool(name="scps", bufs=2, space="PSUM"))  # [128,1024] tiles
        pvps = ctx.enter_context(tc.tile_pool(name="pvps", bufs=1, space="PSUM"))
        if (not causal) and has_mask:
            mkpool = ctx.enter_context(tc.tile_pool(name="mask", bufs=3))

        # ---- constant / weight loads (ordered by first use) --------------
        wqk_t, wro_t, wv_t = [], [], []
        for kb in range(DKB):
            t = wpool.tile([P, 2 * FLOC], pdt, tag=f"wqk{kb}")
            nc.sync.dma_start(t[:], io["wqk"][kb * P:(kb + 1) * P, :])
            wqk_t.append(t)
        if ROPE_PE:
            rotM = wpool.tile([P, P], adt, tag="rotM")
            nc.sync.dma_start(rotM[:], io["rotM"][:])
        else:
            for kb in range(DKB):
                t = wpool.tile([P, 2 * FLOC], pdt, tag=f"wro{kb}")
                nc.sync.dma_start(t[:], io["wro"][kb * P:(kb + 1) * P, :])
                wro_t.append(t)
        # prefetch chunk-0 x tile right after wqk so the first projection
        # matmuls start earlier (one-shot path only: the For_i repeat
        # path reloads xt per iteration as pool slots cycle)
        prefetch_xt = None
        if repeat == 1:
            prefetch_xt = xpool.tile([P, DKB * SC], pdt, tag="xt",
                                     name="xt0p")
            nc.sync.dma_start(
                prefetch_xt[:].rearrange("p (kb s) -> p kb s", kb=DKB),
                io["xT"][:, :, 0:SC])
        cos_t, sin_t = [], []
        for c in range(NCH):
            t = wpool.tile([P, SC], F32, tag=f"cos{c}")
            nc.sync.dma_start(t[:], io["cos2"][:, c * SC:(c + 1) * SC])
            cos_t.append(t)
            t = wpool.tile([P, SC], F32, tag=f"sin{c}")
            nc.sync.dma_start(t[:], io["sin2"][:, c * SC:(c + 1) * SC])
            sin_t.append(t)
        for kb in range(DKB):
            t = wpool.tile([P, FLOC], pdt, tag=f"wv{kb}")
            nc.sync.dma_start(t[:], io["wv"][kb * P:(kb + 1) * P, :])
            wv_t.append(t)
        if causal:
            # 0/1 upper-triangular (k<=q) mask, multiplied into exp(p) on
            # the diagonal blocks (replaces the additive -200 matmul path)
            tri01 = wpool.tile([P, P], adt, tag="tri01")
            nc.sync.dma_start(tri01[:], io["tri01"][:])
        elif has_mask:
            ident = wpool.tile([P, P], adt, tag="ident")
            nc.sync.dma_start(ident[:], io["ident"][:])
        wo_t = []
        for fb in range(FLOC // P):
            t = wpool.tile([P, D], pdt, tag=f"wo{fb}")
            nc.sync.dma_start(t[:], io["wo"][fb * P:(fb + 1) * P, :])
            wo_t.append(t)
        if has_bias:
            bqk_t = wpool.tile([1, 2 * FLOC], F32R, tag="bqk")
            nc.sync.dma_start(bqk_t[:], io["bqk"][:])
            bro_t = wpool.tile([1, 2 * FLOC], F32R, tag="bro")
            nc.sync.dma_start(bro_t[:], io["bro"][:])
            bv_t = wpool.tile([1, FLOC], F32R, tag="bv")
            nc.sync.dma_start(bv_t[:], io["bv"][:])
        ones_f = wpool.tile([1, SC], F32, tag="onesf")
        nc.vector.memset(ones_f[:], 1.0)
        ones_r = wpool.tile([1, SC], F32R, tag="onesr")
        nc.vector.tensor_copy(ones_r[:], ones_f[:])
        onecol_f = wpool.tile([P, 1], F32, tag="onecol")
        nc.vector.memset(onecol_f[:], 1.0)

        # ---- per-chunk emission ------------------------------------------
        # qk tiles: qT_{m}_{c}, kT_{m}_{c}  [128, SC] (m: head pair; head
        # 2m in partitions 0:64, head 2m+1 in 64:128)
        qk_tiles = {}
        v_tiles = {}
        a_tiles = {}

        # v tiles persist (one per key block); their ones-columns (softmax
        # denominator trick) never change, so write them once, outside the
        # repeat loop.
        for s_t in range(KBLK):
            vt = vpool.tile([P, HLOC * (HD + 1)], adt, tag=f"v{s_t}",
                            name=f"vt{s_t}")
            v_tiles[s_t] = vt
            vview = vt[:].rearrange("p (h w) -> p h w", w=HD + 1)
            nc.vector.tensor_copy(
                vview[:, :, HD], onecol_f[:, 0:1].to_broadcast((P, HLOC)))

        def emit_qkv(c):
            """Generator: yields after each natural instruction group so the
            driver can interleave these PE-heavy steps into the ACT-heavy
            attention loop of the previous chunk."""
            if c == 0 and prefetch_xt is not None:
                xt_big = prefetch_xt
            else:
                xt_big = xpool.tile([P, DKB * SC], pdt, tag="xt")
                hv = xt_big[:].rearrange("p (kb s) -> p kb s", kb=DKB)
                hk = DKB // 2    # two dmas -> two DMA engines in parallel
                nc.sync.dma_start(hv[:, :hk, :],
                                  io["xT"][:, :hk, c * SC:(c + 1) * SC])
                nc.sync.dma_start(hv[:, hk:, :],
                                  io["xT"][:, hk:, c * SC:(c + 1) * SC])
            xtv = xt_big[:].rearrange("p (kb s) -> p kb s", kb=DKB)
            xt = [xtv[:, kb, :] for kb in range(DKB)]
            # Q (m=0,1) and K (m=2,3) m-tiles, plus rotated versions
            for m in range(4):
                ps_a = mmps.tile([P, SC], F32, tag="mm")
                for kb in range(DKB):
                    nc.tensor.matmul(
                        ps_a[:], wqk_t[kb][:, m * P:(m + 1) * P], xt[kb],
                        start=(kb == 0), stop=(kb == DKB - 1 and not has_bias))
                if has_bias:
                    nc.tensor.matmul(ps_a[:], bqk_t[0:1, m * P:(m + 1) * P],
                                     ones_r[0:1, :], start=False, stop=True)
                yield
                ps_b = mmps.tile([P, SC], F32, tag="mm")
                if ROPE_PE:
                    # rotate_half(q) = rotM.T @ q: copy q to SBUF (ACT/DVE
                    # alternating), then one permutation matmul
                    qp = tmppool.tile([P, SC], adt, tag="qp")
                    if m % 2 == 0:
                        nc.scalar.activation(qp[:], ps_a[:], AF.Copy)
                    else:
                        nc.vector.tensor_copy(qp[:], ps_a[:])
                    nc.tensor.matmul(ps_b[:], rotM[:], qp[:],
                                     start=True, stop=True)
                else:
                    for kb in range(DKB):
                        nc.tensor.matmul(
                            ps_b[:], wro_t[kb][:, m * P:(m + 1) * P], xt[kb],
                            start=(kb == 0),
                            stop=(kb == DKB - 1 and not has_bias))
                    if has_bias:
                        nc.tensor.matmul(ps_b[:],
                                         bro_t[0:1, m * P:(m + 1) * P],
                                         ones_r[0:1, :], start=False,
                                         stop=True)
                yield
                kind = "qT" if m < 2 else "kT"
                dest = qkpool.tile([P, SC], adt, tag=f"{kind}{m % 2}_{c}")
                qk_tiles[(kind, m % 2, c)] = dest
                t1 = tmppool.tile([P, SC], F32, tag="ropea")
                t2 = tmppool.tile([P, SC], F32, tag="ropeb")
                nc.vector.tensor_mul(t1[:], ps_a[:], cos_t[c][:])
                nc.vector.tensor_mul(t2[:], ps_b[:], sin_t[c][:])
                if GP_ROPE_ADD:
                    nc.gpsimd.tensor_add(dest[:], t1[:], t2[:])
                else:
                    nc.vector.tensor_add(dest[:], t1[:], t2[:])
                yield
            # V for the 4 s-tiles of this chunk, augmented with ones column
            for st in range(SC // P):
                s_t = c * (SC // P) + st
                ps = mmps.tile([P, SC], F32, tag="mm")
                for kb in range(DKB):
                    nc.tensor.matmul(
                        ps[:, :FLOC], xtv[:, kb, st * P:(st + 1) * P],
                        wv_t[kb][:],
                        start=(kb == 0), stop=(kb == DKB - 1 and not has_bias))
                if has_bias:
                    nc.tensor.matmul(ps[:, :FLOC], ones_r[0:1, :P],
                                     bv_t[0:1, :], start=False, stop=True)
                yield
                vview = v_tiles[s_t][:].rearrange("p (h w) -> p h w", w=HD + 1)
                nc.vector.tensor_copy(
                    vview[:, :, :HD],
                    ps[:, :FLOC].rearrange("p (h w) -> p h w", w=HD))
                yield

        def attn_iter_count(c):
            nkb = (4 * c + 4) if causal else KBLK
            return (HLOC // 2) * nkb

        def emit_attn(c):
            """Generator: one yield per (hp, kb) inner step.

            Both heads of a pair are processed together: their score matmuls
            use PE row groups 0:64 / 64:128 (distinct base partitions) so
            the PE overlaps them; exp covers both heads in one activation.
            """
            for hp in range(HLOC // 2):
                at = apool.tile([P, SC], pdt, tag=f"aT{hp}_{c}")
                a_tiles[(hp, c)] = at
                h0, h1 = 2 * hp, 2 * hp + 1
                kbs = list(range(4 * c + 4)) if causal else list(range(KBLK))
                pv0 = pvps.tile([P, SC], F32, tag="pv0")
                pv1 = pvps.tile([P, SC], F32, tag="pv1")
                qt = qk_tiles[("qT", hp, c)]
                for kb in kbs:
                    diag = causal and (kb // 4 == c)
                    qq0 = (kb % 4) * P if diag else 0
                    kt = qk_tiles[("kT", hp, kb // 4)]
                    kcol = slice((kb % 4) * P, (kb % 4 + 1) * P)
                    sc_ps = scps.tile([P, 2 * SC], F32, tag="sc")
                    maskmm = (not causal) and has_mask
                    nc.tensor.matmul(
                        sc_ps[:, qq0:SC], kt[0:HD, kcol], qt[0:HD, qq0:SC],
                        start=True, stop=not maskmm)
                    nc.tensor.matmul(
                        sc_ps[:, SC + qq0:2 * SC], kt[HD:P, kcol],
                        qt[HD:P, qq0:SC], start=True, stop=not maskmm)
                    if maskmm:
                        mk = mkpool.tile([P, SC], adt, tag="mk")
                        nc.sync.dma_start(
                            mk[:], io["maskT"][kb * P:(kb + 1) * P,
                                               c * SC:(c + 1) * SC])
                        nc.tensor.matmul(sc_ps[:, 0:SC], ident[:], mk[:],
                                         start=False, stop=True)
                        nc.tensor.matmul(sc_ps[:, SC:2 * SC], ident[:], mk[:],
                                         start=False, stop=True)
                    pt = ptpool.tile([P, 2 * SC], adt, tag="pt")
                    if qq0 == 0:
                        nc.scalar.activation(pt[:], sc_ps[:], AF.Exp)
                    else:
                        src = sc_ps[:].rearrange("p (two s) -> p two s", s=SC)
                        dst = pt[:].rearrange("p (two s) -> p two s", s=SC)
                        nc.scalar.activation(dst[:, :, qq0:SC],
                                             src[:, :, qq0:SC], AF.Exp)
                    if diag:
                        # zero the strict-lower (k>q) triangle of the 128-col
                        # diagonal sub-block of both heads' halves in one op
                        blk = pt[:].rearrange("p (two s) -> p two s",
                                              s=SC)[:, :, qq0:qq0 + P]
                        tri3 = tri01[:].rearrange(
                            "p (one s) -> p one s", one=1).to_broadcast(
                            (P, 2, P))
                        nc.vector.tensor_mul(blk, blk, tri3)
                    last = kbs[-1]
                    nc.tensor.matmul(
                        pv0[0:HD + 1, qq0:SC],
                        v_tiles[kb][:, h0 * (HD + 1):(h0 + 1) * (HD + 1)],
                        pt[:, qq0:SC],
                        start=(kb == 0), stop=(kb == last))
                    nc.tensor.matmul(
                        pv1[0:HD + 1, qq0:SC],
                        v_tiles[kb][:, h1 * (HD + 1):(h1 + 1) * (HD + 1)],
                        pt[:, SC + qq0:2 * SC],
                        start=(kb == 0), stop=(kb == last))
                    yield
                for hh, pv in ((0, pv0), (HD, pv1)):
                    # copy pv out of PSUM first: frees the accumulation bank
                    # ~2us earlier for the next head-pair's PV matmuls
                    pvs = smpool.tile([HD + 1, SC], F32, tag="pvs")
                    nc.vector.tensor_copy(pvs[:], pv[0:HD + 1, :])
                    recip = smpool.tile([1, SC], F32 if GP_BCAST else F32R,
                                        tag="recip")
                    nc.vector.reciprocal(recip[:], pvs[HD:HD + 1, :])
                    bc = smpool.tile([HD, SC], F32, tag="bc")
                    if GP_BCAST:
                        nc.gpsimd.partition_broadcast(bc[:], recip[0:1, :])
                    else:
                        bc_ps = mmps.tile([P, SC], F32, tag="mm")
                        nc.tensor.matmul(bc_ps[0:HD, :], ones_r[0:1, :HD],
                                         recip[0:1, :], start=True, stop=True)
                        nc.scalar.activation(bc[:], bc_ps[0:HD, :], AF.Copy)
                    nc.vector.tensor_mul(at[hh:hh + HD, :], pvs[0:HD, :],
                                         bc[:])
                yield

        def emit_outproj(c):
            ysb = ypool.tile([P, (SC // P) * D], BF16, tag="ysb")
            for st in range(SC // P):
                for e in range(D // SC):
                    yps = mmps.tile([P, SC], F32, tag="mm")
                    for fb in range(FLOC // P):
                        nc.tensor.matmul(
                            yps[:], a_tiles[(fb, c)][:, st * P:(st + 1) * P],
                            wo_t[fb][:, e * SC:(e + 1) * SC],
                            start=(fb == 0), stop=(fb == FLOC // P - 1))
                    off = st * D + e * SC
                    if e % 2 == 0:
                        nc.scalar.activation(ysb[:, off:off + SC],
                                             yps[:], AF.Copy)
                    else:
                        nc.vector.tensor_copy(ysb[:, off:off + SC],
                                              yps[:])
                    yield
                nc.sync.dma_start(io["y"][c, :, st * D:(st + 1) * D],
                                  ysb[:, st * D:(st + 1) * D])
            yield

        def drain(gen):
            if gen is not None:
                for _ in gen:
                    pass

        def run_interleaved(main_gen, main_len, fillers):
            """Drive main_gen to completion, spreading the filler steps
            (a list of generators, drained in order) evenly across it."""
            total = sum(n for _, n in fillers)
            queue = [g for g, _ in fillers]
            done = 0
            emitted = 0
            for _ in main_gen:
                done += 1
                want = (done * total) // max(main_len, 1)
                while emitted < want and queue:
                    try:
                        next(queue[0])
                        emitted += 1
                    except StopIteration:
                        queue.pop(0)
            for g in queue:
                drain(g)

        QKV_STEPS = 4 * 3 + 4 * 2        # yields in emit_qkv
        OUT_STEPS = 4 * 2 + 1            # yields in emit_outproj

        def emit_all():
            import os as _os
            ablate = _os.environ.get("KERNEL_ABLATE", "")
            if ablate == "proj" and causal:
                # projections + out-proj only (a_tiles read uninitialized)
                drain(emit_qkv(0))
                for c in range(NCH):
                    for hp in range(HLOC // 2):
                        t = apool.tile([P, SC], pdt, tag=f"aT{hp}_{c}",
                                       name=f"abl_a{hp}_{c}")
                        nc.vector.memset(t[:], 0.01)
                        a_tiles[(hp, c)] = t
                    if c + 1 < NCH:
                        drain(emit_qkv(c + 1))
                    drain(emit_outproj(c))
                return
            if ablate == "attn" and causal:
                # attention only: qk/v tiles fabricated by memset
                for c in range(NCH):
                    for kind in ("qT", "kT"):
                        for mm in range(2):
                            t = qkpool.tile([P, SC], adt, tag=f"{kind}{mm}_{c}")
                            nc.vector.memset(t[:], 0.01)
                            qk_tiles[(kind, mm, c)] = t
                    for s_t in range(c * 4, c * 4 + 4):
                        vt = vpool.tile([P, HLOC * (HD + 1)], adt,
                                        tag=f"v{s_t}")
                        nc.vector.memset(vt[:], 0.01)
                        v_tiles[s_t] = vt
                for c in range(NCH):
                    run_interleaved(emit_attn(c), attn_iter_count(c), [])
                return
            if causal:
                drain(emit_qkv(0))
                for c in range(NCH):
                    fillers = []
                    if c + 1 < NCH:
                        fillers.append((emit_qkv(c + 1), QKV_STEPS))
                    if c > 0:
                        fillers.append((emit_outproj(c - 1), OUT_STEPS))
                    run_interleaved(emit_attn(c), attn_iter_count(c), fillers)
                drain(emit_outproj(NCH - 1))
            else:
                # dense attention reads K/V of every chunk: finish all QKV first
                for c in range(NCH):
                    drain(emit_qkv(c))
                for c in range(NCH):
                    drain(emit_attn(c))
                    drain(emit_outproj(c))

        if repeat == 1:
            emit_all()
        else:
            with tc.For_i(0, repeat, 1):
                emit_all()


class _Runner:
    """Cached shard_map+jit executable for one built Bass program.

    Mirrors bass2jax.run_bass_via_pjrt's multi-core path, but reuses the
    traced/jitted function across calls (run_bass_via_pjrt rebuilds it each
    time, costing seconds of retrace per call) and skips output donation
    (this kernel writes every element of y).
    """

    def __init__(self, nc):
        import jax
        import numpy as _np
        from jax.sharding import Mesh, PartitionSpec
        from jax.experimental.shard_map import shard_map
        from concourse import bass2jax as b2j
        from concourse import mybir as mb

        b2j.install_neuronx_cc_hook()
        self.jax = jax
        part_name = (nc.partition_id_tensor.name
                     if nc.partition_id_tensor else None)
        in_names, out_names, out_avals, zero_outs = [], [], [], []
        for alloc in nc.m.functions[0].allocations:
            if not isinstance(alloc, mb.MemoryLocationSet):
                continue
            name = alloc.memorylocations[0].name
            if alloc.kind == "ExternalInput":
                if name != part_name:
                    in_names.append(name)
            elif alloc.kind == "ExternalOutput":
                out_names.append(name)
                out_avals.append(jax.core.ShapedArray(
                    tuple(alloc.tensor_shape), mb.dt.np(alloc.dtype)))
                zero_outs.append(_np.zeros(tuple(alloc.tensor_shape),
                                           mb.dt.np(alloc.dtype)))
        n_params = len(in_names)
        all_names = in_names + out_names
        if part_name is not None:
            all_names = all_names + [part_name]
        self.in_names, self.out_names = in_names, out_names
        self.out_avals = out_avals

        def _body(*args):
            operands = list(args)
            if part_name is not None:
                operands.append(b2j.partition_id_tensor())
            return tuple(b2j._bass_exec_p.bind(
                *operands,
                out_avals=tuple(out_avals),
                in_names=tuple(all_names),
                out_names=tuple(out_names),
                lowering_input_output_aliases=(),
                sim_require_finite=True,
                sim_require_nnan=True,
                nc=nc,
            ))

        self._body = _body
        devices = jax.devices()[:NCORES]
        mesh = Mesh(_np.asarray(devices), ("core",))
        nin = n_params + len(out_names)
        self.fn = jax.jit(shard_map(
            _body, mesh=mesh,
            in_specs=(PartitionSpec("core"),) * nin,
            out_specs=(PartitionSpec("core"),) * len(out_names),
            check_rep=False))
        self.zero_concat = [
            _np.zeros((NCORES * z.shape[0], *z.shape[1:]), z.dtype)
            for z in zero_outs]

    def concat_inputs(self, in_maps):
        import numpy as _np
        return [
            _np.concatenate([_np.asarray(in_maps[c][nm])
                             for c in range(NCORES)], axis=0)
            for nm in self.in_names]

    def run_device(self, dev_args):
        if not hasattr(self, "_zero_dev"):
            self._zero_dev = [self.jax.device_put(z) for z in self.zero_concat]
        out = self.fn(*dev_args, *self._zero_dev)
        self.jax.block_until_ready(out)
        return out

    def time_device(self, dev_args, iters=48, reps=3):
        """Median per-iteration device time: async-dispatch K executions
        (per-device stream serializes), block once; difference vs 1 call."""
        import time as _t
        jax = self.jax
        if not hasattr(self, "_zero_dev"):
            self._zero_dev = [self.jax.device_put(z) for z in self.zero_concat]
        jax.block_until_ready(self.fn(*dev_args, *self._zero_dev))  # warm

        def run_k(k):
            t0 = _t.perf_counter()
            outs = [self.fn(*dev_args, *self._zero_dev) for _ in range(k)]
            jax.block_until_ready(outs)
            return _t.perf_counter() - t0

        est = []
        for _ in range(reps):
            t1 = run_k(1)
            tN = run_k(iters)
            est.append((tN - t1) / (iters - 1))
        est.sort()
        return est[len(est) // 2], est

    def __call__(self, in_maps):
        import numpy as _np
        self._last_concat = self.concat_inputs(in_maps)
        out_arrs = self.fn(*self._last_concat, *self.zero_concat)
        return [
            {nm: _np.asarray(out_arrs[i]).reshape(
                NCORES, *self.out_avals[i].shape)[c]
             for i, nm in enumerate(self.out_names)}
            for c in range(NCORES)
        ]


_RUNNERS: dict = {}


def _get_runner(nc):
    if id(nc) not in _RUNNERS:
        _RUNNERS[id(nc)] = _Runner(nc)
    return _RUNNERS[id(nc)]


def _rope_tables():
    inv_freq = (1.0 / (10000.0 ** (np.arange(0, HD, 2, dtype=np.float32) / HD)))
    t = np.arange(S, dtype=np.float32)
    freqs = np.outer(t, inv_freq).astype(np.float32)      # (S, HD/2)
    emb = np.concatenate([freqs, freqs], axis=-1)          # (S, HD)
    return np.cos(emb).astype(np.float32), np.sin(emb).astype(np.float32)


def _rot_weights(w_loc):
    """rotate_half on the output-feature rows of a local weight slice."""
    r = w_loc.reshape(HLOC, HD, D)
    out = np.concatenate([-r[:, HD // 2:, :], r[:, :HD // 2, :]], axis=1)
    return out.reshape(FLOC, D)


def kernel(x, attn_mask, Wq, bq, Wk, bk, Wv, bv, Wo, bo):
    global _LAST_RESULTS, _LAST_IN_MAPS
    x = np.asarray(x, np.float32)
    attn_mask = np.asarray(attn_mask, np.float32)
    Wq, Wk, Wv, Wo = (np.asarray(w, np.float32) for w in (Wq, Wk, Wv, Wo))
    bq, bk, bv, bo = (np.asarray(b, np.float32) for b in (bq, bk, bv, bo))

    tri = np.tril(np.ones((S, S), dtype=bool))
    causal = bool(np.all(attn_mask[tri] == 0.0)
                  and np.all(attn_mask[~tri] <= -1e8))
    has_mask = bool(np.any(attn_mask != 0.0))
    has_bias = bool(np.any(bq) or np.any(bk) or np.any(bv))

    key = (causal, has_mask, has_bias, PROJ_BF16, ATT_BF16)
    if key not in _CACHE:
        _CACHE[key] = _build(causal, has_mask, has_bias)
    nc = _CACHE[key]
    pnp = ml_dtypes.bfloat16 if PROJ_BF16 else np.float32
    anp = ml_dtypes.bfloat16 if ATT_BF16 else np.float32

    cos, sin = _rope_tables()                 # (S, HD)
    cosT = np.ascontiguousarray(cos.T)        # (HD, S)
    sinT = np.ascontiguousarray(sin.T)
    cos2 = np.concatenate([cosT, cosT], axis=0)   # (128, S)
    sin2 = np.concatenate([sinT, sinT], axis=0)

    scale = 1.0 / np.sqrt(np.float32(HD))
    in_maps = []
    for cid in range(NCORES):
        b, hg = cid // GROUPS, cid % GROUPS
        fs = slice(hg * FLOC, (hg + 1) * FLOC)
        wq_loc = Wq[fs] * scale
        wk_loc = Wk[fs]
        m = {
            "xT": np.ascontiguousarray(
                x[b].T.reshape(DKB, P, S).transpose(1, 0, 2)).astype(pnp),
            "wqk": np.ascontiguousarray(
                np.concatenate([wq_loc, wk_loc], axis=0).T).astype(pnp),
            "wv": np.ascontiguousarray(Wv[fs].T).astype(pnp),
            "wo": np.ascontiguousarray(Wo[:, fs].T).astype(pnp),
            "cos2": cos2,
            "sin2": sin2,
        }
        if ROPE_PE:
            # lhsT for rotate_half: out[i] = -q[i+32] (i%64<32), q[i-32] else
            rm = np.zeros((P, P), dtype=np.float32)
            for o in (0, HD):
                rm[o + HD // 2:o + HD, o:o + HD // 2] = -np.eye(HD // 2)
                rm[o:o + HD // 2, o + HD // 2:o + HD] = np.eye(HD // 2)
            m["rotM"] = rm.astype(anp)
        else:
            m["wro"] = np.ascontiguousarray(
                np.concatenate([_rot_weights(wq_loc), _rot_weights(wk_loc)],
                               axis=0).T).astype(pnp)
        if causal:
            # 1 where k<=q (valid), 0 above: multiplied into exp(scores)
            m["tri01"] = np.triu(np.ones((P, P), dtype=np.float32)).astype(anp)
        elif has_mask:
            m["ident"] = np.eye(P, dtype=np.float32).astype(anp)
            m["maskT"] = np.ascontiguousarray(
                np.maximum(attn_mask.T, NEG).astype(np.float32)).astype(anp)
        if has_bias:
            bq_loc = bq[fs] * scale
            bk_loc = bk[fs]
            m["bqk"] = np.concatenate([bq_loc, bk_loc])[None, :].copy()
            rr = lambda v: np.concatenate(
                [-v.reshape(HLOC, HD)[:, HD // 2:],
                 v.reshape(HLOC, HD)[:, :HD // 2]], axis=1).reshape(-1)
            m["bro"] = np.concatenate([rr(bq_loc), rr(bk_loc)])[None, :].copy()
            m["bv"] = bv[fs][None, :].copy()
        in_maps.append(m)

    _LAST_IN_MAPS = in_maps
    results = _get_runner(nc)(in_maps)
    _LAST_RESULTS = results

    out = np.zeros((B, S, D), dtype=np.float32)
    for cid in range(NCORES):
        yc = results[cid]["y"].astype(np.float32).reshape(NCH, P, SC // P, D)
        out[cid // GROUPS] += yc.transpose(0, 2, 1, 3).reshape(S, D)
    if np.any(bo):
        out += bo[None, None, :]
    return out



# revision 62
# speedup vs baseline: 1.4534x; 1.4534x over previous
"""Causal self-attention (RoPE) Trainium2 Bass kernel, SPMD over 8 NeuronCores.

Sharding: core i -> batch b = i // 4, head group hg = i % 4 (4 heads each).
Per core: QKV projections (f32r matmuls), RoPE via rotated-weight projections,
scores computed transposed [k, q] (softmax denominator via ones-column in V),
causal block skipping, out-proj partial. Host sums the 4 partials per batch.
"""
import numpy as np
import ml_dtypes

import concourse.mybir as mybir
import concourse.tile as tile
from concourse import bacc
from concourse.bass_utils import run_bass_kernel_spmd

B, S, D = 2, 2048, 1024
H, HD = 16, 64
NCORES = 8
GROUPS = NCORES // B          # 4 tensor-parallel cores per batch
HLOC = H // GROUPS            # 4 heads per core
FLOC = HLOC * HD              # 256 local features
P = 128
SC = 512                      # s-chunk (q-chunk) width
NCH = S // SC                 # 4 chunks
KBLK = S // P                 # 16 key blocks
DKB = D // P                  # 8 contraction blocks for projections
NEG = -200.0                  # clamped mask value; exp(-200+s) == 0 in fp32

F32 = mybir.dt.float32
F32R = mybir.dt.float32r
BF16 = mybir.dt.bfloat16
AF = mybir.ActivationFunctionType

# stage dtypes (bf16 streams 2 cols/cycle on PE + fast weight load; f32r is
# ~11-bit-mantissa fp32 at 1 col/cycle)
PROJ_BF16 = True    # xT/wqk/wro/wv/wo/aT (projection + out-proj matmuls)
ATT_BF16 = True     # qT/kT/v_aug/pT/ident/mtri/mask (attention matmuls)
GP_BCAST = True     # softmax-recip broadcast on GpSimd (else PE mm + ACT copy)
GP_ROPE_ADD = False # RoPE t1+t2 add on DVE (gpsimd would force library reloads)
ROPE_PE = True      # rotate_half via one PE permutation matmul instead of a
                    # second full rotated-weight projection (saves 7 MMs/m-tile)

_CACHE: dict = {}
_LAST_RESULTS = None
_LAST_IN_MAPS = None


def _build(causal: bool, has_mask: bool, has_bias: bool, repeat: int = 1):
    nc = bacc.Bacc("TRN2", target_bir_lowering=False, debug=False,
                   num_devices=NCORES)
    dp = nc.declare_dram_parameter
    pdt = BF16 if PROJ_BF16 else F32R
    adt = BF16 if ATT_BF16 else F32R
    io = {
        # x^T permuted [p, kb, s] so each chunk loads with ONE dma
        "xT":   dp("xT",   [P, DKB, S],   pdt, isOutput=False),
        "wqk":  dp("wqk",  [D, 2 * FLOC], pdt, isOutput=False),
        "wv":   dp("wv",   [D, FLOC],     pdt, isOutput=False),
        "wo":   dp("wo",   [FLOC, D],     pdt, isOutput=False),
        "cos2": dp("cos2", [P, S],        F32,  isOutput=False),
        "sin2": dp("sin2", [P, S],        F32,  isOutput=False),
        # y stored [c, p, st*D+d] (token s = c*SC + st*P + p), bf16 partials
        # summed on host in f32
        "y":    dp("y",    [NCH, P, (SC // P) * D], BF16, isOutput=True),
    }
    if ROPE_PE:
        io["rotM"] = dp("rotM", [P, P], adt, isOutput=False)
    else:
        io["wro"] = dp("wro", [D, 2 * FLOC], pdt, isOutput=False)
    if causal:
        io["tri01"] = dp("tri01", [P, P], adt, isOutput=False)
    elif has_mask:
        io["ident"] = dp("ident", [P, P], adt, isOutput=False)
        io["maskT"] = dp("maskT", [S, S], adt, isOutput=False)
    if has_bias:
        io["bqk"] = dp("bqk", [1, 2 * FLOC], F32R, isOutput=False)
        io["bro"] = dp("bro", [1, 2 * FLOC], F32R, isOutput=False)
        io["bv"] = dp("bv", [1, FLOC], F32R, isOutput=False)

    with tile.TileContext(nc) as tc, \
         nc.allow_low_precision(reason="float32r rounding for PE operands"):
        _emit(nc, tc, io, causal, has_mask, has_bias, repeat)
    nc.finalize()
    return nc


def _emit(nc, tc, io, causal, has_mask, has_bias, repeat=1):
    pdt = BF16 if PROJ_BF16 else F32R
    adt = BF16 if ATT_BF16 else F32R
    from contextlib import ExitStack
    ctx = ExitStack()
    with ctx:
        wpool = ctx.enter_context(tc.tile_pool(name="weights", bufs=1))
        xpool = ctx.enter_context(tc.tile_pool(name="xt", bufs=1))
        qkpool = ctx.enter_context(tc.tile_pool(name="qk", bufs=1))
        vpool = ctx.enter_context(tc.tile_pool(name="v", bufs=1))
        apool = ctx.enter_context(tc.tile_pool(name="aT", bufs=1))
        tmppool = ctx.enter_context(tc.tile_pool(name="tmp", bufs=2))
        ptpool = ctx.enter_context(tc.tile_pool(name="pT", bufs=3))
        smpool = ctx.enter_context(tc.tile_pool(name="small", bufs=2))
        ypool = ctx.enter_context(tc.tile_pool(name="y", bufs=2))
        mmps = ctx.enter_context(tc.tile_pool(name="mmps", bufs=2, space="PSUM"))
        scps = ctx.enter_context(tc.tile_pool(name="scps", bufs=2, space="PSUM"))  # [128,1024] tiles
        pvps = ctx.enter_context(tc.tile_pool(name="pvps", bufs=1, space="PSUM"))
        if (not causal) and has_mask:
            mkpool = ctx.enter_context(tc.tile_pool(name="mask", bufs=3))

        # ---- constant / weight loads (ordered by first use) --------------
        wqk_t, wro_t, wv_t = [], [], []
        for kb in range(DKB):
            t = wpool.tile([P, 2 * FLOC], pdt, tag=f"wqk{kb}")
            nc.sync.dma_start(t[:], io["wqk"][kb * P:(kb + 1) * P, :])
            wqk_t.append(t)
        if ROPE_PE:
            rotM = wpool.tile([P, P], adt, tag="rotM")
            nc.sync.dma_start(rotM[:], io["rotM"][:])
        else:
            for kb in range(DKB):
                t = wpool.tile([P, 2 * FLOC], pdt, tag=f"wro{kb}")
                nc.sync.dma_start(t[:], io["wro"][kb * P:(kb + 1) * P, :])
                wro_t.append(t)
        # fixed x tiles, one per chunk (allocated once here, dma'd at the
        # top of every iteration so chunk c+1's x is resident long before
        # its projection filler needs it)
        xt_tiles = []
        for c in range(NCH):
            t = xpool.tile([P, DKB * SC], pdt, tag=f"xt{c}")
            xt_tiles.append(t)
        cos_t, sin_t = [], []
        for c in range(NCH):
            t = wpool.tile([P, SC], F32, tag=f"cos{c}")
            nc.sync.dma_start(t[:], io["cos2"][:, c * SC:(c + 1) * SC])
            cos_t.append(t)
            t = wpool.tile([P, SC], F32, tag=f"sin{c}")
            nc.sync.dma_start(t[:], io["sin2"][:, c * SC:(c + 1) * SC])
            sin_t.append(t)
        for kb in range(DKB):
            t = wpool.tile([P, FLOC], pdt, tag=f"wv{kb}")
            nc.sync.dma_start(t[:], io["wv"][kb * P:(kb + 1) * P, :])
            wv_t.append(t)
        if causal:
            # 0/1 upper-triangular (k<=q) mask, multiplied into exp(p) on
            # the diagonal blocks (replaces the additive -200 matmul path)
            tri01 = wpool.tile([P, P], adt, tag="tri01")
            nc.sync.dma_start(tri01[:], io["tri01"][:])
        elif has_mask:
            ident = wpool.tile([P, P], adt, tag="ident")
            nc.sync.dma_start(ident[:], io["ident"][:])
        wo_t = []
        for fb in range(FLOC // P):
            t = wpool.tile([P, D], pdt, tag=f"wo{fb}")
            nc.sync.dma_start(t[:], io["wo"][fb * P:(fb + 1) * P, :])
            wo_t.append(t)
        if has_bias:
            bqk_t = wpool.tile([1, 2 * FLOC], F32R, tag="bqk")
            nc.sync.dma_start(bqk_t[:], io["bqk"][:])
            bro_t = wpool.tile([1, 2 * FLOC], F32R, tag="bro")
            nc.sync.dma_start(bro_t[:], io["bro"][:])
            bv_t = wpool.tile([1, FLOC], F32R, tag="bv")
            nc.sync.dma_start(bv_t[:], io["bv"][:])
        ones_f = wpool.tile([1, SC], F32, tag="onesf")
        nc.vector.memset(ones_f[:], 1.0)
        ones_r = wpool.tile([1, SC], F32R, tag="onesr")
        nc.vector.tensor_copy(ones_r[:], ones_f[:])
        onecol_f = wpool.tile([P, 1], F32, tag="onecol")
        nc.vector.memset(onecol_f[:], 1.0)

        # ---- per-chunk emission ------------------------------------------
        # qk tiles: qT_{m}_{c}, kT_{m}_{c}  [128, SC] (m: head pair; head
        # 2m in partitions 0:64, head 2m+1 in 64:128)
        qk_tiles = {}
        v_tiles = {}
        a_tiles = {}

        # v tiles persist (one per key block); their ones-columns (softmax
        # denominator trick) never change, so write them once, outside the
        # repeat loop.
        for s_t in range(KBLK):
            vt = vpool.tile([P, HLOC * (HD + 1)], adt, tag=f"v{s_t}",
                            name=f"vt{s_t}")
            v_tiles[s_t] = vt
            vview = vt[:].rearrange("p (h w) -> p h w", w=HD + 1)
            nc.vector.tensor_copy(
                vview[:, :, HD], onecol_f[:, 0:1].to_broadcast((P, HLOC)))

        def emit_qkv(c):
            """Generator: yields after each natural instruction group so the
            driver can interleave these PE-heavy steps into the ACT-heavy
            attention loop of the previous chunk."""
            xtv = xt_tiles[c][:].rearrange("p (kb s) -> p kb s", kb=DKB)
            xt = [xtv[:, kb, :] for kb in range(DKB)]
            # Q (m=0,1) and K (m=2,3) m-tiles, plus rotated versions
            for m in range(4):
                ps_a = mmps.tile([P, SC], F32, tag="mm")
                for kb in range(DKB):
                    nc.tensor.matmul(
                        ps_a[:], wqk_t[kb][:, m * P:(m + 1) * P], xt[kb],
                        start=(kb == 0), stop=(kb == DKB - 1 and not has_bias))
                if has_bias:
                    nc.tensor.matmul(ps_a[:], bqk_t[0:1, m * P:(m + 1) * P],
                                     ones_r[0:1, :], start=False, stop=True)
                yield
                ps_b = mmps.tile([P, SC], F32, tag="mm")
                if ROPE_PE:
                    # rotate_half(q) = rotM.T @ q: copy q to SBUF (ACT/DVE
                    # alternating), then one permutation matmul
                    qp = tmppool.tile([P, SC], adt, tag="qp")
                    if m % 2 == 0:
                        nc.scalar.activation(qp[:], ps_a[:], AF.Copy)
                    else:
                        nc.vector.tensor_copy(qp[:], ps_a[:])
                    nc.tensor.matmul(ps_b[:], rotM[:], qp[:],
                                     start=True, stop=True)
                else:
                    for kb in range(DKB):
                        nc.tensor.matmul(
                            ps_b[:], wro_t[kb][:, m * P:(m + 1) * P], xt[kb],
                            start=(kb == 0),
                            stop=(kb == DKB - 1 and not has_bias))
                    if has_bias:
                        nc.tensor.matmul(ps_b[:],
                                         bro_t[0:1, m * P:(m + 1) * P],
                                         ones_r[0:1, :], start=False,
                                         stop=True)
                yield
                kind = "qT" if m < 2 else "kT"
                dest = qkpool.tile([P, SC], adt, tag=f"{kind}{m % 2}_{c}")
                qk_tiles[(kind, m % 2, c)] = dest
                t1 = tmppool.tile([P, SC], F32, tag="ropea")
                t2 = tmppool.tile([P, SC], F32, tag="ropeb")
                nc.vector.tensor_mul(t1[:], ps_a[:], cos_t[c][:])
                nc.vector.tensor_mul(t2[:], ps_b[:], sin_t[c][:])
                if GP_ROPE_ADD:
                    nc.gpsimd.tensor_add(dest[:], t1[:], t2[:])
                else:
                    nc.vector.tensor_add(dest[:], t1[:], t2[:])
                yield
            # V for the 4 s-tiles of this chunk, augmented with ones column
            for st in range(SC // P):
                s_t = c * (SC // P) + st
                ps = mmps.tile([P, SC], F32, tag="mm")
                for kb in range(DKB):
                    nc.tensor.matmul(
                        ps[:, :FLOC], xtv[:, kb, st * P:(st + 1) * P],
                        wv_t[kb][:],
                        start=(kb == 0), stop=(kb == DKB - 1 and not has_bias))
                if has_bias:
                    nc.tensor.matmul(ps[:, :FLOC], ones_r[0:1, :P],
                                     bv_t[0:1, :], start=False, stop=True)
                yield
                vview = v_tiles[s_t][:].rearrange("p (h w) -> p h w", w=HD + 1)
                nc.vector.tensor_copy(
                    vview[:, :, :HD],
                    ps[:, :FLOC].rearrange("p (h w) -> p h w", w=HD))
                yield

        def attn_iter_count(c):
            nkb = (4 * c + 4) if causal else KBLK
            return (HLOC // 2) * nkb

        def emit_attn(c):
            """Generator: one yield per (hp, kb) inner step.

            Both heads of a pair are processed together: their score matmuls
            use PE row groups 0:64 / 64:128 (distinct base partitions) so
            the PE overlaps them; exp covers both heads in one activation.
            """
            for hp in range(HLOC // 2):
                at = apool.tile([P, SC], pdt, tag=f"aT{hp}_{c}")
                a_tiles[(hp, c)] = at
                h0, h1 = 2 * hp, 2 * hp + 1
                kbs = list(range(4 * c + 4)) if causal else list(range(KBLK))
                pv0 = pvps.tile([P, SC], F32, tag="pv0")
                pv1 = pvps.tile([P, SC], F32, tag="pv1")
                qt = qk_tiles[("qT", hp, c)]
                for kb in kbs:
                    diag = causal and (kb // 4 == c)
                    qq0 = (kb % 4) * P if diag else 0
                    kt = qk_tiles[("kT", hp, kb // 4)]
                    kcol = slice((kb % 4) * P, (kb % 4 + 1) * P)
                    sc_ps = scps.tile([P, 2 * SC], F32, tag="sc")
                    maskmm = (not causal) and has_mask
                    nc.tensor.matmul(
                        sc_ps[:, qq0:SC], kt[0:HD, kcol], qt[0:HD, qq0:SC],
                        start=True, stop=not maskmm)
                    nc.tensor.matmul(
                        sc_ps[:, SC + qq0:2 * SC], kt[HD:P, kcol],
                        qt[HD:P, qq0:SC], start=True, stop=not maskmm)
                    if maskmm:
                        mk = mkpool.tile([P, SC], adt, tag="mk")
                        nc.sync.dma_start(
                            mk[:], io["maskT"][kb * P:(kb + 1) * P,
                                               c * SC:(c + 1) * SC])
                        nc.tensor.matmul(sc_ps[:, 0:SC], ident[:], mk[:],
                                         start=False, stop=True)
                        nc.tensor.matmul(sc_ps[:, SC:2 * SC], ident[:], mk[:],
                                         start=False, stop=True)
                    pt = ptpool.tile([P, 2 * SC], adt, tag="pt")
                    if qq0 == 0:
                        nc.scalar.activation(pt[:], sc_ps[:], AF.Exp)
                    else:
                        src = sc_ps[:].rearrange("p (two s) -> p two s", s=SC)
                        dst = pt[:].rearrange("p (two s) -> p two s", s=SC)
                        nc.scalar.activation(dst[:, :, qq0:SC],
                                             src[:, :, qq0:SC], AF.Exp)
                    if diag:
                        # zero the strict-lower (k>q) triangle of the 128-col
                        # diagonal sub-block of both heads' halves in one op
                        blk = pt[:].rearrange("p (two s) -> p two s",
                                              s=SC)[:, :, qq0:qq0 + P]
                        tri3 = tri01[:].rearrange(
                            "p (one s) -> p one s", one=1).to_broadcast(
                            (P, 2, P))
                        nc.vector.tensor_mul(blk, blk, tri3)
                    last = kbs[-1]
                    nc.tensor.matmul(
                        pv0[0:HD + 1, qq0:SC],
                        v_tiles[kb][:, h0 * (HD + 1):(h0 + 1) * (HD + 1)],
                        pt[:, qq0:SC],
                        start=(kb == 0), stop=(kb == last))
                    nc.tensor.matmul(
                        pv1[0:HD + 1, qq0:SC],
                        v_tiles[kb][:, h1 * (HD + 1):(h1 + 1) * (HD + 1)],
                        pt[:, SC + qq0:2 * SC],
                        start=(kb == 0), stop=(kb == last))
                    yield
                for hh, pv in ((0, pv0), (HD, pv1)):
                    # copy pv out of PSUM first: frees the accumulation bank
                    # ~2us earlier for the next head-pair's PV matmuls
                    pvs = smpool.tile([HD + 1, SC], F32, tag="pvs")
                    nc.vector.tensor_copy(pvs[:], pv[0:HD + 1, :])
                    recip = smpool.tile([1, SC], F32 if GP_BCAST else F32R,
                                        tag="recip")
                    nc.vector.reciprocal(recip[:], pvs[HD:HD + 1, :])
                    bc = smpool.tile([HD, SC], F32, tag="bc")
                    if GP_BCAST:
                        nc.gpsimd.partition_broadcast(bc[:], recip[0:1, :])
                    else:
                        bc_ps = mmps.tile([P, SC], F32, tag="mm")
                        nc.tensor.matmul(bc_ps[0:HD, :], ones_r[0:1, :HD],
                                         recip[0:1, :], start=True, stop=True)
                        nc.scalar.activation(bc[:], bc_ps[0:HD, :], AF.Copy)
                    nc.vector.tensor_mul(at[hh:hh + HD, :], pvs[0:HD, :],
                                         bc[:])
                yield

        def emit_outproj(c):
            ysb = ypool.tile([P, (SC // P) * D], BF16, tag="ysb")
            for st in range(SC // P):
                for e in range(D // SC):
                    yps = mmps.tile([P, SC], F32, tag="mm")
                    for fb in range(FLOC // P):
                        nc.tensor.matmul(
                            yps[:], a_tiles[(fb, c)][:, st * P:(st + 1) * P],
                            wo_t[fb][:, e * SC:(e + 1) * SC],
                            start=(fb == 0), stop=(fb == FLOC // P - 1))
                    off = st * D + e * SC
                    if e % 2 == 0:
                        nc.scalar.activation(ysb[:, off:off + SC],
                                             yps[:], AF.Copy)
                    else:
                        nc.vector.tensor_copy(ysb[:, off:off + SC],
                                              yps[:])
                    yield
                nc.sync.dma_start(io["y"][c, :, st * D:(st + 1) * D],
                                  ysb[:, st * D:(st + 1) * D])
            yield

        def drain(gen):
            if gen is not None:
                for _ in gen:
                    pass

        def run_interleaved(main_gen, main_len, fillers):
            """Drive main_gen to completion, spreading the filler steps
            (a list of generators, drained round-robin) evenly across it."""
            total = sum(n for _, n in fillers)
            queue = [g for g, _ in fillers]
            done = 0
            emitted = 0
            rr = 0
            for _ in main_gen:
                done += 1
                want = (done * total) // max(main_len, 1)
                while emitted < want and queue:
                    rr %= len(queue)
                    try:
                        next(queue[rr])
                        emitted += 1
                        rr += 1
                    except StopIteration:
                        queue.pop(rr)
            for g in queue:
                drain(g)

        QKV_STEPS = 4 * 3 + 4 * 2        # yields in emit_qkv
        OUT_STEPS = 4 * 2 + 1            # yields in emit_outproj

        def emit_xt_dmas():
            # all chunks' x loads issued up front: two dmas per chunk land
            # on parallel DMA engines and complete well before each chunk's
            # projection filler consumes them
            hk = DKB // 2
            for c in range(NCH):
                hv = xt_tiles[c][:].rearrange("p (kb s) -> p kb s", kb=DKB)
                nc.sync.dma_start(hv[:, :hk, :],
                                  io["xT"][:, :hk, c * SC:(c + 1) * SC])
                nc.sync.dma_start(hv[:, hk:, :],
                                  io["xT"][:, hk:, c * SC:(c + 1) * SC])

        def emit_all():
            emit_xt_dmas()
            if causal:
                drain(emit_qkv(0))
                for c in range(NCH):
                    fillers = []
                    if c + 1 < NCH:
                        fillers.append((emit_qkv(c + 1), QKV_STEPS))
                    if c > 0:
                        fillers.append((emit_outproj(c - 1), OUT_STEPS))
                    run_interleaved(emit_attn(c), attn_iter_count(c), fillers)
                drain(emit_outproj(NCH - 1))
            else:
                # dense attention reads K/V of every chunk: finish all QKV first
                for c in range(NCH):
                    drain(emit_qkv(c))
                for c in range(NCH):
                    drain(emit_attn(c))
                    drain(emit_outproj(c))

        if repeat == 1:
            emit_all()
        else:
            with tc.For_i(0, repeat, 1):
                emit_all()


class _Runner:
    """Cached shard_map+jit executable for one built Bass program.

    Mirrors bass2jax.run_bass_via_pjrt's multi-core path, but reuses the
    traced/jitted function across calls (run_bass_via_pjrt rebuilds it each
    time, costing seconds of retrace per call) and skips output donation
    (this kernel writes every element of y).
    """

    def __init__(self, nc):
        import jax
        import numpy as _np
        from jax.sharding import Mesh, PartitionSpec
        from jax.experimental.shard_map import shard_map
        from concourse import bass2jax as b2j
        from concourse import mybir as mb

        b2j.install_neuronx_cc_hook()
        self.jax = jax
        part_name = (nc.partition_id_tensor.name
                     if nc.partition_id_tensor else None)
        in_names, out_names, out_avals, zero_outs = [], [], [], []
        for alloc in nc.m.functions[0].allocations:
            if not isinstance(alloc, mb.MemoryLocationSet):
                continue
            name = alloc.memorylocations[0].name
            if alloc.kind == "ExternalInput":
                if name != part_name:
                    in_names.append(name)
            elif alloc.kind == "ExternalOutput":
                out_names.append(name)
                out_avals.append(jax.core.ShapedArray(
                    tuple(alloc.tensor_shape), mb.dt.np(alloc.dtype)))
                zero_outs.append(_np.zeros(tuple(alloc.tensor_shape),
                                           mb.dt.np(alloc.dtype)))
        n_params = len(in_names)
        all_names = in_names + out_names
        if part_name is not None:
            all_names = all_names + [part_name]
        self.in_names, self.out_names = in_names, out_names
        self.out_avals = out_avals

        def _body(*args):
            operands = list(args)
            if part_name is not None:
                operands.append(b2j.partition_id_tensor())
            return tuple(b2j._bass_exec_p.bind(
                *operands,
                out_avals=tuple(out_avals),
                in_names=tuple(all_names),
                out_names=tuple(out_names),
                lowering_input_output_aliases=(),
                sim_require_finite=True,
                sim_require_nnan=True,
                nc=nc,
            ))

        self._body = _body
        devices = jax.devices()[:NCORES]
        mesh = Mesh(_np.asarray(devices), ("core",))
        nin = n_params + len(out_names)
        self.fn = jax.jit(shard_map(
            _body, mesh=mesh,
            in_specs=(PartitionSpec("core"),) * nin,
            out_specs=(PartitionSpec("core"),) * len(out_names),
            check_rep=False))
        self.zero_concat = [
            _np.zeros((NCORES * z.shape[0], *z.shape[1:]), z.dtype)
            for z in zero_outs]

    def concat_inputs(self, in_maps):
        import numpy as _np
        return [
            _np.concatenate([_np.asarray(in_maps[c][nm])
                             for c in range(NCORES)], axis=0)
            for nm in self.in_names]

    def run_device(self, dev_args):
        if not hasattr(self, "_zero_dev"):
            self._zero_dev = [self.jax.device_put(z) for z in self.zero_concat]
        out = self.fn(*dev_args, *self._zero_dev)
        self.jax.block_until_ready(out)
        return out

    def time_device(self, dev_args, iters=48, reps=3):
        """Median per-iteration device time: async-dispatch K executions
        (per-device stream serializes), block once; difference vs 1 call."""
        import time as _t
        jax = self.jax
        if not hasattr(self, "_zero_dev"):
            self._zero_dev = [self.jax.device_put(z) for z in self.zero_concat]
        jax.block_until_ready(self.fn(*dev_args, *self._zero_dev))  # warm

        def run_k(k):
            t0 = _t.perf_counter()
            outs = [self.fn(*dev_args, *self._zero_dev) for _ in range(k)]
            jax.block_until_ready(outs)
            return _t.perf_counter() - t0

        est = []
        for _ in range(reps):
            t1 = run_k(1)
            tN = run_k(iters)
            est.append((tN - t1) / (iters - 1))
        est.sort()
        return est[len(est) // 2], est

    def __call__(self, in_maps):
        import numpy as _np
        self._last_concat = self.concat_inputs(in_maps)
        out_arrs = self.fn(*self._last_concat, *self.zero_concat)
        return [
            {nm: _np.asarray(out_arrs[i]).reshape(
                NCORES, *self.out_avals[i].shape)[c]
             for i, nm in enumerate(self.out_names)}
            for c in range(NCORES)
        ]


_RUNNERS: dict = {}


def _get_runner(nc):
    if id(nc) not in _RUNNERS:
        _RUNNERS[id(nc)] = _Runner(nc)
    return _RUNNERS[id(nc)]


def _rope_tables():
    inv_freq = (1.0 / (10000.0 ** (np.arange(0, HD, 2, dtype=np.float32) / HD)))
    t = np.arange(S, dtype=np.float32)
    freqs = np.outer(t, inv_freq).astype(np.float32)      # (S, HD/2)
    emb = np.concatenate([freqs, freqs], axis=-1)          # (S, HD)
    return np.cos(emb).astype(np.float32), np.sin(emb).astype(np.float32)


def _rot_weights(w_loc):
    """rotate_half on the output-feature rows of a local weight slice."""
    r = w_loc.reshape(HLOC, HD, D)
    out = np.concatenate([-r[:, HD // 2:, :], r[:, :HD // 2, :]], axis=1)
    return out.reshape(FLOC, D)


def kernel(x, attn_mask, Wq, bq, Wk, bk, Wv, bv, Wo, bo):
    global _LAST_RESULTS, _LAST_IN_MAPS
    x = np.asarray(x, np.float32)
    attn_mask = np.asarray(attn_mask, np.float32)
    Wq, Wk, Wv, Wo = (np.asarray(w, np.float32) for w in (Wq, Wk, Wv, Wo))
    bq, bk, bv, bo = (np.asarray(b, np.float32) for b in (bq, bk, bv, bo))

    tri = np.tril(np.ones((S, S), dtype=bool))
    causal = bool(np.all(attn_mask[tri] == 0.0)
                  and np.all(attn_mask[~tri] <= -1e8))
    has_mask = bool(np.any(attn_mask != 0.0))
    has_bias = bool(np.any(bq) or np.any(bk) or np.any(bv))

    key = (causal, has_mask, has_bias, PROJ_BF16, ATT_BF16)
    if key not in _CACHE:
        _CACHE[key] = _build(causal, has_mask, has_bias)
    nc = _CACHE[key]
    pnp = ml_dtypes.bfloat16 if PROJ_BF16 else np.float32
    anp = ml_dtypes.bfloat16 if ATT_BF16 else np.float32

    cos, sin = _rope_tables()                 # (S, HD)
    cosT = np.ascontiguousarray(cos.T)        # (HD, S)
    sinT = np.ascontiguousarray(sin.T)
    cos2 = np.concatenate([cosT, cosT], axis=0)   # (128, S)
    sin2 = np.concatenate([sinT, sinT], axis=0)

    scale = 1.0 / np.sqrt(np.float32(HD))
    in_maps = []
    for cid in range(NCORES):
        b, hg = cid // GROUPS, cid % GROUPS
        fs = slice(hg * FLOC, (hg + 1) * FLOC)
        wq_loc = Wq[fs] * scale
        wk_loc = Wk[fs]
        m = {
            "xT": np.ascontiguousarray(
                x[b].T.reshape(DKB, P, S).transpose(1, 0, 2)).astype(pnp),
            "wqk": np.ascontiguousarray(
                np.concatenate([wq_loc, wk_loc], axis=0).T).astype(pnp),
            "wv": np.ascontiguousarray(Wv[fs].T).astype(pnp),
            "wo": np.ascontiguousarray(Wo[:, fs].T).astype(pnp),
            "cos2": cos2,
            "sin2": sin2,
        }
        if ROPE_PE:
            # lhsT for rotate_half: out[i] = -q[i+32] (i%64<32), q[i-32] else
            rm = np.zeros((P, P), dtype=np.float32)
            for o in (0, HD):
                rm[o + HD // 2:o + HD, o:o + HD // 2] = -np.eye(HD // 2)
                rm[o:o + HD // 2, o + HD // 2:o + HD] = np.eye(HD // 2)
            m["rotM"] = rm.astype(anp)
        else:
            m["wro"] = np.ascontiguousarray(
                np.concatenate([_rot_weights(wq_loc), _rot_weights(wk_loc)],
                               axis=0).T).astype(pnp)
        if causal:
            # 1 where k<=q (valid), 0 above: multiplied into exp(scores)
            m["tri01"] = np.triu(np.ones((P, P), dtype=np.float32)).astype(anp)
        elif has_mask:
            m["ident"] = np.eye(P, dtype=np.float32).astype(anp)
            m["maskT"] = np.ascontiguousarray(
                np.maximum(attn_mask.T, NEG).astype(np.float32)).astype(anp)
        if has_bias:
            bq_loc = bq[fs] * scale
            bk_loc = bk[fs]
            m["bqk"] = np.concatenate([bq_loc, bk_loc])[None, :].copy()
            rr = lambda v: np.concatenate(
                [-v.reshape(HLOC, HD)[:, HD // 2:],
                 v.reshape(HLOC, HD)[:, :HD // 2]], axis=1).reshape(-1)
            m["bro"] = np.concatenate([rr(bq_loc), rr(bk_loc)])[None, :].copy()
            m["bv"] = bv[fs][None, :].copy()
        in_maps.append(m)

    _LAST_IN_MAPS = in_maps
    results = _get_runner(nc)(in_maps)
    _LAST_RESULTS = results

    out = np.zeros((B, S, D), dtype=np.float32)
    for cid in range(NCORES):
        yc = results[cid]["y"].astype(np.float32).reshape(NCH, P, SC // P, D)
        out[cid // GROUPS] += yc.transpose(0, 2, 1, 3).reshape(S, D)
    if np.any(bo):
        out += bo[None, None, :]
    return out

